# revision 9
# baseline (speedup 1.0000x reference)
"""BiLSTM (B=64, L=256, D=512, H=512) on 8 Trainium2 NeuronCores.

v3: each core runs TWO independent recurrences interleaved — fw and bw for
the same 8-row batch slice (8 cores x 8 rows = 64).  The two per-step
dependency chains (matmul -> sigmoid -> cell update -> transpose) are
independent, so one recurrence's scalar/vector chain hides behind the
other's PE streaming and vice versa.

All matmul operands bf16 (fp32 PSUM accumulate).  g-gate weights are
pre-scaled by 2 on the host so tanh(x) = 2*sigmoid(2x) - 1 lets ONE sigmoid
activation cover all four gate strips.
"""

import numpy as np
import ml_dtypes

from concourse import tile, mybir, bacc
from concourse.bass_utils import run_bass_kernel_spmd
from concourse.masks import make_identity

FP = mybir.dt.float32
BF = mybir.dt.bfloat16
AF = mybir.ActivationFunctionType
ALU = mybir.AluOpType

B2 = 8        # local batch per recurrence (2 recurrences per core)
L = 256       # timesteps
D = 512       # input dim
H = 512       # hidden
NG = 4 * H    # gate width
TOK = L * B2  # tokens per recurrence
NM = TOK // 128

_CACHED_NC = None


def _build():
    nc = bacc.Bacc("TRN2", target_bir_lowering=False, debug=False)

    xT = nc.dram_tensor("xT", [2, D, TOK], BF, kind="ExternalInput").ap()
    W = nc.dram_tensor("W", [2, D + H, NG], BF, kind="ExternalInput").ap()
    bias = nc.dram_tensor("bias", [2, NG], BF, kind="ExternalInput").ap()
    out_h = nc.dram_tensor("out_h", [2, L, B2, H], BF,
                           kind="ExternalOutput").ap()
    xp_dram = nc.dram_tensor("xp_dram", [2, NM, 128, NG], BF).ap()

    with tile.TileContext(nc, trace_sim=False) as tc:
        with tc.tile_pool(name="wpool", bufs=1) as wpool, \
             tc.tile_pool(name="cpool", bufs=1) as cpool:
            W_t = [[], []]
            for d in range(2):
                for k in range(8):
                    wt = wpool.tile([128, NG], BF, tag=f"w{d}_{k}",
                                    name=f"w{d}_{k}")
                    nc.sync.dma_start(wt[:], W[d, 128 * k:128 * (k + 1), :])
                    W_t[d].append(wt)
            bias_t = []
            for d in range(2):
                bt = wpool.tile([1, NG], BF, tag=f"bias{d}", name=f"bias{d}")
                nc.sync.dma_start(bt[:], bias[d:d + 1, :])
                bias_t.append(bt)
            ones_t = cpool.tile([1, 128], BF)
            nc.vector.memset(ones_t[:, :], 1.0)
            ident = cpool.tile([B2, B2], BF)
            make_identity(nc, ident[:, :])

            # Phase 1: xpart GEMMs (both directions)
            with tc.tile_pool(name="p1x", bufs=3) as p1x, \
                 tc.tile_pool(name="p1ps", bufs=4, space="PSUM") as p1ps:
                for d in range(2):
                    for m in range(NM):
                        xm = p1x.tile([128, 4, 128], BF, tag="xm", name="xm")
                        for k in range(4):
                            nc.sync.dma_start(
                                xm[:, k, :],
                                xT[d, 128 * k:128 * (k + 1),
                                   128 * m:128 * (m + 1)])
                        for n in range(4):
                            ps = p1ps.tile([128, 512], FP, tag="ps1",
                                           name="ps1")
                            for k in range(4):
                                nc.tensor.matmul(
                                    ps[:, :], xm[:, k, :],
                                    W_t[d][k][:, 512 * n:512 * (n + 1)],
                                    start=(k == 0), stop=False)
                            nc.tensor.matmul(
                                ps[:, :], ones_t[:, :],
                                bias_t[d][:, 512 * n:512 * (n + 1)],
                                start=False, stop=True)
                            sb = p1x.tile([128, 512], BF, tag="sb1",
                                          name="sb1")
                            if n % 2 == 0:
                                nc.scalar.copy(sb[:, :], ps[:, :])
                            else:
                                nc.vector.tensor_copy(sb[:, :], ps[:, :])
                            nc.sync.dma_start(
                                xp_dram[d, m, :, 512 * n:512 * (n + 1)],
                                sb[:, :])

            # Phase 2: interleaved recurrences
            with tc.tile_pool(name="xpp", bufs=4) as xpp, \
                 tc.tile_pool(name="st", bufs=2) as st, \
                 tc.tile_pool(name="ch", bufs=2) as ch, \
                 tc.tile_pool(name="gps", bufs=4, space="PSUM") as gps, \
                 tc.tile_pool(name="tps", bufs=2, space="PSUM") as tps:

                c_prev = []
                hT_prev = []
                for d in range(2):
                    c0 = st.tile([B2, H], BF, tag=f"c{d}", name=f"c{d}_0")
                    nc.vector.memset(c0[:, :], 0.0)
                    hT0 = st.tile([128, 32], BF, tag=f"hT{d}",
                                  name=f"hT{d}_0")
                    nc.vector.memset(hT0[:, :], 0.0)
                    c_prev.append(c0)
                    hT_prev.append(hT0)

                for t in range(L):
                    P = []
                    for d in range(2):
                        xp_t = xpp.tile([B2, NG], BF, tag=f"xp{d}",
                                        name=f"xp{d}")
                        nc.sync.dma_start(
                            xp_t[:],
                            xp_dram[d, t // 16,
                                    B2 * (t % 16):B2 * (t % 16) + B2, :])

                        Pd = gps.tile([128, 512], FP, tag="P", name=f"P{d}")
                        for j in range(4):
                            nc.tensor.matmul(
                                Pd[32 * j:32 * j + B2, :], ident[:, :],
                                xp_t[:, 512 * j:512 * (j + 1)],
                                start=True, stop=False,
                                tile_position=(0, 32 * j))
                        for k in range(4):
                            for j in range(4):
                                nc.tensor.matmul(
                                    Pd[32 * j:32 * j + B2, :],
                                    hT_prev[d][:, 8 * k:8 * (k + 1)],
                                    W_t[d][4 + k][:, 512 * j:512 * (j + 1)],
                                    start=False, stop=(k == 3),
                                    tile_position=(0, 32 * j))
                        P.append(Pd)

                    for d in range(2):
                        Pd = P[d]
                        # strips: f@0:8, i@32:40, o@64:72, g~@96:104
                        s = ch.tile([104, H], BF, tag=f"s{d}", name=f"s{d}")
                        nc.scalar.activation(s[:, :], Pd[0:104, :],
                                             AF.Sigmoid)
                        u = ch.tile([40, H], BF, tag=f"u{d}", name=f"u{d}")
                        nc.vector.tensor_scalar(
                            u[32:40, :], s[96:104, :], 2.0, -1.0,
                            op0=ALU.mult, op1=ALU.add)
                        t1 = ch.tile([B2, H], BF, tag=f"t1{d}", name=f"t1{d}")
                        nc.vector.tensor_mul(t1[:, :], s[0:B2, :],
                                             c_prev[d][:, :])
                        t2 = ch.tile([B2, H], BF, tag=f"t2{d}", name=f"t2{d}")
                        nc.vector.tensor_mul(t2[:, :], s[32:40, :],
                                             u[32:40, :])
                        c_new = st.tile([B2, H], BF, tag=f"c{d}",
                                        name=f"c{d}")
                        nc.vector.tensor_add(c_new[:, :], t1[:, :], t2[:, :])
                        th = ch.tile([72, H], BF, tag=f"th{d}", name=f"th{d}")
                        nc.scalar.activation(th[64:72, :], c_new[:, :],
                                             AF.Tanh)
                        h_new = st.tile([B2, H], BF, tag=f"h{d}",
                                        name=f"h{d}")
                        nc.vector.tensor_mul(h_new[:, :], s[64:72, :],
                                             th[64:72, :])

                        nc.sync.dma_start(out_h[d, t, :, :], h_new[:, :])

                        pst = tps.tile([128, 32], BF, tag="pst",
                                       name=f"pst{d}")
                        for kc in range(4):
                            nc.tensor.transpose(
                                pst[:, 8 * kc:8 * (kc + 1)],
                                h_new[:, 128 * kc:128 * (kc + 1)],
                                ident[:, :])
                        hT_new = st.tile([128, 32], BF, tag=f"hT{d}",
                                         name=f"hT{d}")
                        nc.scalar.copy(hT_new[:, :], pst[:, :])
                        c_prev[d] = c_new
                        hT_prev[d] = hT_new
    nc.compile()
    return nc


def _host_prepare(x_full, weights, bslice):
    xs_f = x_full[bslice]
    xs_b = xs_f[:, ::-1, :]
    xT = np.stack([
        np.ascontiguousarray(xs_f.transpose(2, 1, 0).reshape(D, TOK)),
        np.ascontiguousarray(xs_b.transpose(2, 1, 0).reshape(D, TOK)),
    ])
    Wc = []
    bc = []
    for dname in ("fw", "bw"):
        w = np.concatenate(
            [weights[f"W_{dname}_{n}"].T for n in "fiog"], axis=1).copy()
        b = np.concatenate(
            [weights[f"b_{dname}_{n}"] for n in "fiog"]).copy()
        # tanh fold: g strip pre-activations scaled by 2
        w[:, 3 * H:] *= 2.0
        b[3 * H:] *= 2.0
        Wc.append(w)
        bc.append(b)
    return {"xT": np.ascontiguousarray(xT).astype(ml_dtypes.bfloat16),
            "W": np.ascontiguousarray(np.stack(Wc)).astype(ml_dtypes.bfloat16),
            "bias": np.ascontiguousarray(np.stack(bc)).astype(
                ml_dtypes.bfloat16)}


def kernel(**inputs):
    global _CACHED_NC
    inputs = {k: np.asarray(v) for k, v in inputs.items()}
    x = inputs["x"]
    Bx, Lx, _ = x.shape
    assert (Bx, Lx) == (64, L)

    if _CACHED_NC is None:
        _CACHED_NC = _build()
    nc = _CACHED_NC

    in_maps = []
    for ci in range(8):
        bs = ci * B2
        in_maps.append(_host_prepare(x, inputs, slice(bs, bs + B2)))

    res = run_bass_kernel_spmd(nc, in_maps, core_ids=list(range(8)))

    hf = np.zeros((L, Bx, H), np.float32)
    hb = np.zeros((L, Bx, H), np.float32)
    for ci in range(8):
        bs = ci * B2
        oh = np.asarray(res.results[ci]["out_h"]).astype(np.float32)
        hf[:, bs:bs + B2, :] = oh[0]
        hb[:, bs:bs + B2, :] = oh[1][::-1]

    # faithful to the reference: stack time-major, flatten, hstack, reshape
    flat = np.concatenate([hf.reshape(-1, H), hb.reshape(-1, H)], axis=1)
    return flat.reshape(Bx, Lx, 2 * H).astype(np.float32)


# revision 13
# speedup vs baseline: 1.3244x; 1.3244x over previous
"""BiLSTM (B=64, L=256, D=512, H=512) on 8 Trainium2 NeuronCores.

Strategy: 8 cores = 2 directions x 4 batch-slices of 16 (weights replicated
per direction, sequential time loop local to each core — no cross-core
communication).  Backward-direction cores receive time-reversed x, so every
core runs the identical SPMD program.

v2 (bf16): all matmul operands in bf16 (4x faster PE streaming than fp32,
single HW pass), fp32 PSUM accumulate.  The g-gate weights are pre-scaled by
2 on the host so tanh(x) = 2*sigmoid(2x) - 1 lets ONE sigmoid activation
cover all four gate strips; the 2s-1 affine runs as a single DVE
tensor_scalar op.

Per-core program:
  Phase 1: xpart[token, 4H] = x_t @ Wx.T + bias as one large GEMM, bf16 out,
           staged to DRAM.
  Phase 2: 256 recurrence steps.  Per step:
    - gates psum tile (128, 512) holds 4 gate strips f/i/o/g at partition
      offsets {0,32,64,96} via column-tiled matmuls (tile_position);
    - one Sigmoid on ScalarE over rows 0:112; cell/hidden update on VectorE;
    - h (16, 512) transposed back to hT (128, 64) with 4 PE-transposes and
      ONE batched psum->sbuf copy on ScalarE.
"""

import numpy as np
import ml_dtypes

from concourse import tile, mybir, bacc
from concourse.bass_utils import run_bass_kernel_spmd
from concourse.masks import make_identity

FP = mybir.dt.float32
BF = mybir.dt.bfloat16
AF = mybir.ActivationFunctionType
ALU = mybir.AluOpType

B = 16        # local batch per core
L = 256       # timesteps
D = 512       # input dim
H = 512       # hidden
NG = 4 * H    # gate width
TOK = L * B   # tokens per core
NM = TOK // 128

_CACHED_NC = None


def _build():
    nc = bacc.Bacc("TRN2", target_bir_lowering=False, debug=False)

    xT = nc.dram_tensor("xT", [D, TOK], BF, kind="ExternalInput").ap()
    W = nc.dram_tensor("W", [D + H, NG], BF, kind="ExternalInput").ap()
    bias = nc.dram_tensor("bias", [1, NG], BF, kind="ExternalInput").ap()
    out_h = nc.dram_tensor("out_h", [L, B, H], BF, kind="ExternalOutput").ap()
    xp_dram = nc.dram_tensor("xp_dram", [NM, 128, NG], BF).ap()

    with tile.TileContext(nc, trace_sim=False) as tc:
        with tc.tile_pool(name="wpool", bufs=1) as wpool, \
             tc.tile_pool(name="cpool", bufs=1) as cpool:
            W_t = []
            for k in range(8):
                wt = wpool.tile([128, NG], BF, tag=f"w{k}", name=f"w{k}")
                nc.sync.dma_start(wt[:], W[128 * k:128 * (k + 1), :])
                W_t.append(wt)
            bias_t = wpool.tile([1, NG], BF)
            nc.sync.dma_start(bias_t[:], bias[:, :])
            ones_t = cpool.tile([1, 128], BF)
            nc.vector.memset(ones_t[:, :], 1.0)
            ident = cpool.tile([B, B], BF)
            make_identity(nc, ident[:, :])

            # Phase 1: xpart GEMM
            with tc.tile_pool(name="p1x", bufs=3) as p1x, \
                 tc.tile_pool(name="p1ps", bufs=4, space="PSUM") as p1ps:
                for m in range(NM):
                    xm = p1x.tile([128, 4, 128], BF, tag="xm", name="xm")
                    for k in range(4):
                        nc.sync.dma_start(
                            xm[:, k, :],
                            xT[128 * k:128 * (k + 1), 128 * m:128 * (m + 1)])
                    for n in range(4):
                        ps = p1ps.tile([128, 512], FP, tag="ps1", name="ps1")
                        for k in range(4):
                            nc.tensor.matmul(
                                ps[:, :], xm[:, k, :],
                                W_t[k][:, 512 * n:512 * (n + 1)],
                                start=(k == 0), stop=False)
                        nc.tensor.matmul(
                            ps[:, :], ones_t[:, :],
                            bias_t[:, 512 * n:512 * (n + 1)],
                            start=False, stop=True)
                        sb = p1x.tile([128, 512], BF, tag="sb1", name="sb1")
                        if n % 2 == 0:
                            nc.scalar.copy(sb[:, :], ps[:, :])
                        else:
                            nc.vector.tensor_copy(sb[:, :], ps[:, :])
                        nc.sync.dma_start(
                            xp_dram[m, :, 512 * n:512 * (n + 1)], sb[:, :])

            # Phase 2: recurrence
            with tc.tile_pool(name="xpp", bufs=4) as xpp, \
                 tc.tile_pool(name="st", bufs=2) as st, \
                 tc.tile_pool(name="ch", bufs=2) as ch, \
                 tc.tile_pool(name="gps", bufs=2, space="PSUM") as gps, \
                 tc.tile_pool(name="tps", bufs=2, space="PSUM") as tps:

                c_prev = st.tile([B, H], BF, tag="c", name="c0")
                nc.vector.memset(c_prev[:, :], 0.0)
                hT_prev = []
                for half in range(2):
                    h0 = st.tile([128, 32], BF, tag=f"hT{half}",
                                 name=f"hT{half}_0")
                    nc.vector.memset(h0[:, :], 0.0)
                    hT_prev.append(h0)

                for t in range(L):
                    xp_t = xpp.tile([B, NG], BF, tag="xp", name="xp")
                    nc.sync.dma_start(
                        xp_t[:],
                        xp_dram[t // 8, B * (t % 8):B * (t % 8) + B, :])

                    P = gps.tile([128, 512], FP, tag="P", name="P")
                    for j in range(4):
                        nc.tensor.matmul(
                            P[32 * j:32 * j + B, :], ident[:, :],
                            xp_t[:, 512 * j:512 * (j + 1)],
                            start=True, stop=False, tile_position=(0, 32 * j))
                    for k in range(4):
                        for j in range(4):
                            nc.tensor.matmul(
                                P[32 * j:32 * j + B, :],
                                hT_prev[k // 2][:, 16 * (k % 2):
                                                16 * (k % 2) + B],
                                W_t[4 + k][:, 512 * j:512 * (j + 1)],
                                start=False, stop=(k == 3),
                                tile_position=(0, 32 * j))

                    # strips: f@0:16, i@32:48, o@64:80, g~@96:112 (g~ = sig(2x))
                    s = ch.tile([112, H], BF, tag="s", name="s")
                    nc.scalar.activation(s[:, :], P[0:112, :], AF.Sigmoid)
                    # u = 2*g~ - 1 = tanh(x_g), placed at rows 32:48 to align
                    # with the i strip
                    u = ch.tile([48, H], BF, tag="u", name="u")
                    nc.vector.tensor_scalar(
                        u[32:48, :], s[96:112, :], 2.0, -1.0,
                        op0=ALU.mult, op1=ALU.add)
                    t1 = ch.tile([B, H], BF, tag="t1", name="t1")
                    nc.vector.tensor_mul(t1[:, :], s[0:B, :], c_prev[:, :])
                    t2 = ch.tile([B, H], BF, tag="t2", name="t2")
                    nc.vector.tensor_mul(t2[:, :], s[32:48, :], u[32:48, :])
                    c_new = st.tile([B, H], BF, tag="c", name="c")
                    nc.vector.tensor_add(c_new[:, :], t1[:, :], t2[:, :])

                    # tail split in two hidden-halves so next-step k-rounds
                    # 0,1 start as soon as the first half's hT is ready
                    th = ch.tile([80, H], BF, tag="th", name="th")
                    h_half = []
                    hT_new = []
                    for half in range(2):
                        c0, c1 = 256 * half, 256 * (half + 1)
                        nc.scalar.activation(
                            th[64:80, c0:c1], c_new[:, c0:c1], AF.Tanh)
                        hh = st.tile([B, 256], BF, tag=f"h{half}",
                                     name=f"h{half}")
                        nc.vector.tensor_mul(
                            hh[:, :], s[64:80, c0:c1], th[64:80, c0:c1])
                        h_half.append(hh)
                        nc.sync.dma_start(out_h[t, :, c0:c1], hh[:, :])

                        pst = tps.tile([128, 32], BF, tag=f"pst{half}",
                                       name=f"pst{half}")
                        for kc in range(2):
                            nc.tensor.transpose(
                                pst[:, 16 * kc:16 * (kc + 1)],
                                hh[:, 128 * kc:128 * (kc + 1)], ident[:, :])
                        hTn = st.tile([128, 32], BF, tag=f"hT{half}",
                                      name=f"hT{half}")
                        nc.scalar.copy(hTn[:, :], pst[:, :])
                        hT_new.append(hTn)

                    c_prev = c_new
                    hT_prev = hT_new
    nc.compile()
    return nc


def _host_prepare(x_full, weights, direction, bslice):
    xs = x_full[bslice]
    if direction == "bw":
        xs = xs[:, ::-1, :]
    xT = np.ascontiguousarray(xs.transpose(2, 1, 0).reshape(D, TOK))
    Wc = np.concatenate(
        [weights[f"W_{direction}_{n}"].T for n in "fiog"], axis=1).copy()
    bc = np.concatenate(
        [weights[f"b_{direction}_{n}"] for n in "fiog"])[None, :].copy()
    # tanh fold: g strip pre-activations scaled by 2 (tanh(x) = 2*sig(2x)-1)
    Wc[:, 3 * H:] *= 2.0
    bc[:, 3 * H:] *= 2.0
    return {"xT": np.ascontiguousarray(xT).astype(ml_dtypes.bfloat16),
            "W": np.ascontiguousarray(Wc).astype(ml_dtypes.bfloat16),
            "bias": np.ascontiguousarray(bc).astype(ml_dtypes.bfloat16)}


def kernel(**inputs):
    global _CACHED_NC
    inputs = {k: np.asarray(v) for k, v in inputs.items()}
    x = inputs["x"]
    Bx, Lx, _ = x.shape
    assert (Bx, Lx) == (64, L)

    if _CACHED_NC is None:
        _CACHED_NC = _build()
    nc = _CACHED_NC

    in_maps = []
    meta = []
    for ci in range(8):
        d = "fw" if ci < 4 else "bw"
        bs = (ci % 4) * B
        in_maps.append(_host_prepare(x, inputs, d, slice(bs, bs + B)))
        meta.append((d, bs))

    res = run_bass_kernel_spmd(nc, in_maps, core_ids=list(range(8)))

    hf = np.zeros((L, Bx, H), np.float32)
    hb = np.zeros((L, Bx, H), np.float32)
    for ci in range(8):
        d, bs = meta[ci]
        oh = np.asarray(res.results[ci]["out_h"]).astype(np.float32)
        if d == "fw":
            hf[:, bs:bs + B, :] = oh
        else:
            hb[:, bs:bs + B, :] = oh[::-1]

    # faithful to the reference: stack time-major, flatten, hstack, reshape
    flat = np.concatenate([hf.reshape(-1, H), hb.reshape(-1, H)], axis=1)
    return flat.reshape(Bx, Lx, 2 * H).astype(np.float32)
